# revision 20
# baseline (speedup 1.0000x reference)
"""Trainium2 Bass kernel for nn_EquivariantAttentionLayer.

Reference computation (N=128 frames, P=256 points, D=128, OUT=256, HEADS=16, HD=16):
  qkv  = einsum('ijd,qdhm->qhmij', x, W1)         # temporal QKV
  s1   = einsum('abij,abIj->aiIj', q, k); a1 = softmax(s1, axis=I)
  t    = einsum('aiIj,abIj->abij', a1, v)
  qkv2 = einsum('hmij,qhmgn->qgnij', t, W2)       # point QKV (mix over both head axes)
  s2   = einsum('abij,abiJ->aijJ', q2, k2); a2 = softmax(s2, axis=J)
  pa   = einsum('aijJ,abiJ->ijab', a2, v2).reshape(N,P,256)
  out  = (pa @ fc1_w + fc1_b) @ fc2_w + fc2_b     # NO nonlinearity -> collapses to one 256x256 matmul

Sharding: phase A is point-sharded (temporal attention is independent per point),
phase B/C are frame-sharded (point attention is independent per frame). Two
half-sized AllToAlls re-shard t from point-shards to frame-shards. Phase A
interleaves QKV prep with attention in 8-point chunks and ships each chunk's
slice as soon as it drains, so AllToAll #1 launches ~40% in. Phase B computes
the [J-half0, j-half0] quadrant of point attention (partial sums + partial Z
in SBUF) while AllToAll #2 is in flight. Engine roles: ACT does only the
softmax exps; DVE does all PSUM evictions (batched wide); Pool (gpsimd, no
PSUM access) does the SBUF-side normalizes and memsets. The FC pair is
collapsed on the host: Wc = fc1_w @ fc2_w ; bc = fc1_b @ fc2_w + fc2_b.
Points are processed in a permuted order (j' = hc*128 + s*16 + jc16); the host
un-permutes the output rows. Heads are processed in PERM order; the host
permutes W2/Wc rows to match.
"""

import numpy as np

# ---- problem dims (hardcoded) ----
NF, NP, D = 128, 256, 128       # frames (i/I), points (j/J), input dim
A_, B_ = 16, 16                 # HD (a/g), HEADS (b/n)
F = A_ * B_                     # 256 features
NCORE = 8
PC = NP // NCORE                # 32 points per core (phase A)
HC = PC // 2                    # 16 points per exchange half
NI = NF // NCORE                # 16 frames per core (phase B)
TOK = NF * PC                   # 4096 tokens per core (both phases)

# Head-processing order: batch bh handles PE row groups {2bh, 2bh+1} so that
# same-PSUM-bank score matmuls are always same-group (HW: cross-group same-bank
# PE writes are fatal).
PERM = [4 * (k // 2) + 2 * bh + (k % 2) for bh in range(2) for k in range(8)]

# Point order as seen by phase B / the raw device output (host un-permutes).
JPERM = np.array([s * PC + hc * HC + jc
                  for hc in range(2) for s in range(NCORE) for jc in range(HC)])


def build_program(phases="AB", n_cores=NCORE, reps=1):
    """Build the SPMD Bass program. phases in {"AB", "A", "B"} (A/B for testing).
    reps>1 repeats the whole body (for wall-clock delta timing)."""
    import concourse.bacc as bacc
    import concourse.mybir as mybir
    import concourse.tile as tile
    from concourse.masks import make_identity

    dt = mybir.dt
    f32 = dt.float32
    f32r = dt.float32r
    f16 = dt.float16

    nc = bacc.Bacc(None, target_bir_lowering=False, num_devices=n_cores)

    if "A" in phases:
        x_d = nc.dram_tensor("x", [NF, PC, D], f16, kind="ExternalInput")
        w1qk_d = nc.dram_tensor("w1qk", [D, 8 * 128], f16, kind="ExternalInput")
        w1v_d = nc.dram_tensor("w1v", [D, F], f16, kind="ExternalInput")
    if "B" in phases:
        w2qk_d = nc.dram_tensor("w2qk", [F, 8 * 128], f16, kind="ExternalInput")
        w2v_d = nc.dram_tensor("w2v", [F, F], f16, kind="ExternalInput")
        wc_d = nc.dram_tensor("wc", [F, F], f32r, kind="ExternalInput")
        bc_d = nc.dram_tensor("bc", [128, F], f32, kind="ExternalInput")
        out_d = nc.dram_tensor("out", [NI, NP, F], f32, kind="ExternalOutput")

    # exchange buffers (per half): tsh[s, f, jc16, il] = t[f, i=s*NI+il, jc]
    kindA = "ExternalOutput" if phases == "A" else None
    kindB = "ExternalInput" if phases == "B" else None
    tsh_ds = tex_ds = None
    if "A" in phases:
        tsh_ds = [[nc.dram_tensor(f"tsh{r}_{h}", [NCORE, F, HC, NI], f16,
                                  **({"kind": kindA} if kindA else {}))
                   for h in range(2)] for r in range(reps)]
    if phases == "AB":
        tex_ds = [[nc.dram_tensor(f"tex{r}_{h}", [NCORE, F, HC, NI], f16)
                   for h in range(2)] for r in range(reps)]
    elif phases == "B":
        tex_ds = [[nc.dram_tensor(f"tex0_{h}", [NCORE, F, HC, NI], f16,
                                  kind="ExternalInput") for h in range(2)]]

    with tile.TileContext(nc) as tc:
        with tc.tile_pool(name="consts", bufs=1) as consts:
            ident = consts.tile([128, 128], f32, tag="ident")
            make_identity(nc, ident[:])

            for r in range(reps):
                if "A" in phases:
                    def do_coll(h, _r=r):
                        if phases != "AB":
                            return
                        nc.gpsimd.collective_compute(
                            "AllToAll", mybir.AluOpType.bypass,
                            replica_groups=[list(range(n_cores))],
                            ins=[tsh_ds[_r][h][:]], outs=[tex_ds[_r][h][:]])
                    _phase_a(nc, tc, tsh_ds[r], do_coll, x_d, w1qk_d, w1v_d,
                             ident, mybir)
                if "B" in phases:
                    _phase_b(nc, tc, out_d, tex_ds[min(r, len(tex_ds) - 1)],
                             w2qk_d, w2v_d, wc_d, bc_d, ident, mybir)

    nc.compile()
    return nc


def _phase_a(nc, tc, tsh_d, do_coll, x_d, w1qk_d, w1v_d, ident, mybir):
    """Temporal QKV + temporal attention for this core's PC points.

    Prep (transpose + QKV mix) and attention interleave in 8-point chunks;
    each chunk's t-slice ships (partial tsh DMA) as soon as it drains."""
    dt = mybir.dt
    f32, f16, bf16 = dt.float32, dt.float16, dt.bfloat16
    Exp = mybir.ActivationFunctionType.Exp
    MUL = mybir.AluOpType.mult

    with tc.tile_pool(name="a_sb", bufs=1) as sb, \
         tc.tile_pool(name="a_ld", bufs=1) as ld, \
         tc.tile_pool(name="a_exp", bufs=3) as expp, \
         tc.tile_pool(name="a_pss", bufs=2, space="PSUM") as pss, \
         tc.tile_pool(name="a_psv", bufs=4, space="PSUM") as psv:

        w1qk_sb = sb.tile([128, 8 * 128], f16, tag="w1qk")
        nc.sync.dma_start(w1qk_sb[:], w1qk_d[:])
        w1v_sb = sb.tile([128, F], f16, tag="w1v")
        nc.sync.dma_start(w1v_sb[:], w1v_d[:])

        # xt_all[d, j*128+i] = x[i, j, d]; loaded transposed via DMA xbar
        xt_all = sb.tile([128, TOK], f16, tag="xt")
        for j in range(PC):
            nc.sync.dma_start_transpose(xt_all[:, j * 128:(j + 1) * 128],
                                        x_d[:, j, :])

        # Q/K, one tile, c-major: chunk c (of 8) holds a in [4c,4c+4), partition
        # 32*(a%4)+b (rows +16..31 zero), free = (c, j, i). c 0-3 = q, 4-7 = k.
        qk = sb.tile([128, 8 * TOK], f16, tag="qk")
        qk_r = qk[:].rearrange("p (c t) -> p c t", c=8)
        # V^T (+ones col): vt[i, (j, a, 17)]; col 16 of each (j,a) block is 1.0
        vt = sb.tile([128, PC * A_ * 17], bf16, tag="vt")
        vt_r = vt[:].rearrange("p (j a c) -> p j a c", j=PC, a=A_, c=17)
        nc.gpsimd.memset(vt_r[:, :, :, 16:17], 1.0)

        Copy = mybir.ActivationFunctionType.Copy
        ei = [0]

        def _mx(nt, cp):
            ps = pss.tile([128, 1024], f32, tag="s", name="mx")
            for ci in range(2):
                nc.tensor.matmul(
                    ps[:, ci * 512:(ci + 1) * 512],
                    w1qk_sb[:, (cp * 2 + ci) * 128:(cp * 2 + ci + 1) * 128],
                    xt_all[:, nt * 512:(nt + 1) * 512],
                    start=True, stop=True)
            dst = qk_r[:, cp * 2:cp * 2 + 2, nt * 512:(nt + 1) * 512]
            srcv = ps[:].rearrange("p (c t) -> p c t", c=2)
            if ei[0] % 4 == 3:
                nc.scalar.activation(dst, srcv, Copy)
            else:
                nc.vector.tensor_copy(dst, srcv)
            ei[0] += 1

        def _vx(pc, g):
            ps = psv.tile([128, 512], f32, tag="w", bufs=2, name="vx")
            for q in range(2):
                j = pc * 8 + g * 2 + q
                nc.tensor.matmul(ps[:, q * F:(q + 1) * F],
                                 xt_all[:, j * 128:(j + 1) * 128],
                                 w1v_sb[:], start=True, stop=True)
            j0 = pc * 8 + g * 2
            nc.vector.tensor_copy(
                vt_r[:, j0:j0 + 2, :, 0:16],
                ps[:].rearrange("p (g a b) -> p g a b", g=2, a=A_))

        def prep_thunks(pc):
            return ([lambda nt=nt, cp=cp: _mx(nt, cp)
                       for nt in (pc * 2, pc * 2 + 1) for cp in range(4)]
                    + [lambda g=g: _vx(pc, g) for g in range(4)])

        def prep_chunk(pc):
            for th in prep_thunks(pc):
                th()

        # temporal attention; tu[i, (j, pos, b)] unnormalized (pos = PERM'd
        # head order); zb[i, (j, pos)] holds Z.
        tu = sb.tile([128, PC * F], f32, tag="tu")
        tu_r = tu[:].rearrange("p (j a b) -> p j a b", j=PC, a=A_)
        zb = sb.tile([128, PC * A_], f32, tag="zb")
        zb_r = zb[:].rearrange("p (j a) -> p j a", j=PC)
        # tab[h]: feature-major t; free = (jh, s, jc16, il) [tsh slice order]
        tab = [sb.tile([128, TOK], f16, tag=f"tab{h}", name=f"tab{h}")
               for h in range(2)]
        colof = lambda k: (k % 2) * 512 + (k // 2) * 128  # bank = row group

        def emit_scores1(j, bh):
            sps = pss.tile([128, 1024], f32, tag="s", name="sps")
            for k in range(8):
                a = PERM[bh * 8 + k]
                c, s4 = a // 4, a % 4
                tp = (96, 0) if s4 == 3 else None
                # S'[I, i]: lhsT=K (b,I), rhs=Q (b,i)
                nc.tensor.matmul(
                    sps[:, colof(k):colof(k) + 128],
                    qk_r[32 * s4:32 * s4 + 16, 4 + c, j * 128:(j + 1) * 128],
                    qk_r[32 * s4:32 * s4 + 16, c, j * 128:(j + 1) * 128],
                    start=True, stop=True, tile_position=tp)
            return sps

        tps_map = {}

        def process_a(j, bh, sps):
            if bh == 0:
                tps_map[j] = psv.tile([128, 512], f32, tag="v", bufs=2,
                                      name="tps")
            tps = tps_map[j]
            aex = expp.tile([128, 1024], bf16, tag="aex", name="aex")
            nc.scalar.activation(aex[:], sps[:], Exp)
            for k in range(8):
                a = PERM[bh * 8 + k]
                # t^T[i, (b,Z)] = A'^T @ [V^T | 1]
                nc.tensor.matmul(tps[:, bh * 136 + k * 17: bh * 136 + k * 17 + 17],
                                 aex[:, colof(k):colof(k) + 128],
                                 vt_r[:, j, a, :], start=True, stop=True)
            if bh == 1:
                tr = tps_map.pop(j)[:, 0:272] \
                    .rearrange("p (s c) -> p s c", s=16)
                nc.vector.tensor_copy(tu_r[:, j, :, :], tr[:, :, 0:16])
                nc.vector.tensor_copy(zb_r[:, j, :], tr[:, :, 16])
                if j % 8 == 7:
                    tail_chunk(j // 8)

        def tail_chunk(ch):
            """Normalize chunk ch's 8 points, transpose to feature-major,
            ship the chunk's tsh slice; A2A after each half's last chunk."""
            jh, jo = ch // 2, ch * 8
            zv = zb_r[:, jo:jo + 8, :]
            nc.vector.reciprocal(zv, zv)
            nc.gpsimd.tensor_tensor(
                tu_r[:, jo:jo + 8, :, :], tu_r[:, jo:jo + 8, :, :],
                zv.rearrange("p j a -> p j a ()").broadcast_to(
                    [128, 8, A_, B_]), op=MUL)
            # transpose to feature-major: per h, 2 quads of 4 points
            tab_v = [tab[h][:, jh * 2048:(jh + 1) * 2048]
                     .rearrange("p (s j i) -> p s j i", s=NCORE, j=HC)
                     for h in range(2)]
            for h in range(2):
                for g in range(2):
                    ps = psv.tile([128, 512], f32, tag="w", bufs=2, name="tt")
                    for q in range(4):
                        j = jo + g * 4 + q
                        nc.tensor.transpose(ps[:, q * 128:(q + 1) * 128],
                                            tu[:, j * F + h * 128:
                                               j * F + (h + 1) * 128],
                                            ident[:])
                    jc0 = (ch % 2) * 8 + g * 4
                    nc.vector.tensor_copy(
                        tab_v[h][:, :, jc0:jc0 + 4, :],
                        ps[:].rearrange("p (j s i) -> p s j i", j=4, s=NCORE))
            jr = slice((ch % 2) * 8, (ch % 2) * 8 + 8)
            for h in range(2):
                nc.sync.dma_start(
                    tsh_d[jh][:, h * 128:(h + 1) * 128, jr, :]
                        .rearrange("s f j i -> f s (j i)"),
                    tab_v[h][:, :, jr, :].rearrange("p s j i -> p s (j i)"))
            if ch % 2 == 1:
                do_coll(jh)

        from collections import deque
        fill = deque()
        prep_chunk(0)
        prev = None
        for ch in range(4):
            if ch + 1 < 4:
                fill.extend(prep_thunks(ch + 1))
            for jc in range(8):
                j = ch * 8 + jc
                for bh in range(2):
                    sps = emit_scores1(j, bh)
                    if prev is not None:
                        process_a(*prev)
                    prev = (j, bh, sps)
                    if fill:
                        fill.popleft()()
        process_a(*prev)


def _phase_b(nc, tc, out_d, tex_d, w2qk_d, w2v_d, wc_d, bc_d, ident, mybir):
    """Point mix + point attention + collapsed FC for this core's NI frames.

    The [J half0, j half0] quadrant of the attention runs right after half 0
    arrives (partial sums + partial Z in SBUF), overlapping AllToAll #2.
    Token order is (il, j') with j' = hc*128 + s*16 + jc16 (host un-permutes)."""
    from collections import deque
    dt = mybir.dt
    f32, f16, bf16, f32r = dt.float32, dt.float16, dt.bfloat16, dt.float32r
    Exp = mybir.ActivationFunctionType.Exp
    MUL = mybir.AluOpType.mult
    ADD = mybir.AluOpType.add

    with tc.tile_pool(name="b_sb", bufs=1) as sb, \
         tc.tile_pool(name="b_exp", bufs=3) as expp, \
         tc.tile_pool(name="b_out", bufs=3) as outp, \
         tc.tile_pool(name="b_pss", bufs=2, space="PSUM") as pss, \
         tc.tile_pool(name="b_psv", bufs=4, space="PSUM") as psv:

        w2v_sb = sb.tile([128, 2 * F], f16, tag="w2v")  # col block kt = rows kt*128..
        nc.sync.dma_start(w2v_sb[:, 0:F], w2v_d[0:128, :])
        nc.sync.dma_start(w2v_sb[:, F:2 * F], w2v_d[128:256, :])
        wc_sb = sb.tile([128, 2 * F], f32r, tag="wc")
        nc.sync.dma_start(wc_sb[:, 0:F], wc_d[0:128, :])
        nc.sync.dma_start(wc_sb[:, F:2 * F], wc_d[128:256, :])
        bias_sb = sb.tile([128, F], f32, tag="bias")
        nc.sync.dma_start(bias_sb[:], bc_d[:])
        w2qk_sb = [sb.tile([128, 1024], f16, tag=f"w2qk{kt}",
                           name=f"w2qk{kt}") for kt in range(2)]
        for kt in range(2):
            nc.sync.dma_start(w2qk_sb[kt][:], w2qk_d[kt * 128:(kt + 1) * 128, :])

        # q2/k2, one tile, c-major; free = (c, hc, s, jc16, il)
        q2k2 = sb.tile([128, 8 * TOK], f16, tag="q2k2")
        q2_r = q2k2[:].rearrange("p (c hc s j i) -> p c hc s j i",
                                 c=8, hc=2, s=NCORE, j=HC)
        # v2t[hc][j'_loc, (il, a, 17)]
        v2t = [sb.tile([128, NI * A_ * 17], bf16, tag=f"v2t{h}", name=f"v2t{h}")
               for h in range(2)]
        v2t_r = [v2t[h][:].rearrange("p (i a c) -> p i a c", i=NI, a=A_)
                 for h in range(2)]
        for h in range(2):
            nc.gpsimd.memset(v2t_r[h][:, :, :, 16:17], 1.0)
        # t2[h][f_local, (hc, s, jc16, il)]
        t2 = [sb.tile([128, TOK], f16, tag=f"t2_{h}", name=f"t2_{h}")
              for h in range(2)]
        t2_r = [t2[h][:].rearrange("p (hc s j i) -> p hc s j i",
                                   hc=2, s=NCORE, j=HC) for h in range(2)]

        def _t2load(hc):
            for h in range(2):
                for s in range(NCORE):
                    nc.sync.dma_start(
                        t2_r[h][:, hc, s, :, :],
                        tex_d[hc][s, h * 128:(h + 1) * 128, :, :])

        def _mx2(hc, nt, cp):
            ps = pss.tile([128, 1024], f32, tag="s", name="mx2")
            for ci in range(2):
                c = cp * 2 + ci
                for kt in range(2):
                    nc.tensor.matmul(
                        ps[:, ci * 512:(ci + 1) * 512],
                        w2qk_sb[kt][:, c * 128:(c + 1) * 128],
                        t2[kt][:, hc * 2048 + nt * 512:
                               hc * 2048 + (nt + 1) * 512],
                        start=(kt == 0), stop=(kt == 1))
            nc.vector.tensor_copy(
                q2_r[:, cp * 2:cp * 2 + 2, hc, nt * 2:(nt + 1) * 2, :, :]
                    .rearrange("p c s j i -> p c (s j i)"),
                ps[:].rearrange("p (c t) -> p c t", c=2))

        def _v2x(hc, ip):
            ps = psv.tile([128, 512], f32, tag="w", bufs=2, name="v2x")
            for q in range(2):
                il = ip * 2 + q
                for kt in range(2):
                    nc.tensor.matmul(
                        ps[:, q * F:(q + 1) * F],
                        t2_r[kt][:, hc, :, :, il],
                        w2v_sb[:, kt * F:(kt + 1) * F],
                        start=(kt == 0), stop=(kt == 1))
            nc.vector.tensor_copy(
                v2t_r[hc][:, ip * 2:ip * 2 + 2, :, 0:16],
                ps[:].rearrange("p (g a b) -> p g a b", g=2, a=A_))

        def mix_thunks(hc):
            return ([lambda: _t2load(hc)]
                    + [lambda nt=nt, cp=cp: _mx2(hc, nt, cp)
                       for nt in range(4) for cp in range(4)]
                    + [lambda ip=ip: _v2x(hc, ip) for ip in range(NI // 2)])

        def load_and_mix(hc):
            for th in mix_thunks(hc):
                th()

        # point attention: pa_tok[jh][j'_loc, (il, pos, b)] unnorm; z2 = Z
        pa_tok = [sb.tile([128, NI * F], f32, tag=f"pat{jh}",
                          name=f"pat{jh}") for jh in range(2)]
        pa_r = [pa_tok[jh][:].rearrange("p (i a b) -> p i a b", i=NI, a=A_)
                for jh in range(2)]
        z2 = [sb.tile([128, NI * A_], f32, tag=f"z2_{jh}", name=f"z2_{jh}")
              for jh in range(2)]
        z2_r = [z2[jh][:].rearrange("p (i a) -> p i a", i=NI)
                for jh in range(2)]
        colof = lambda k: (k % 2) * 512 + (k // 2) * 128

        def emit_scores2(il, bh, Jh, jh):
            # 8 heads x [J' = half Jh (128 keys), j = half jh (128 queries)]
            sps = pss.tile([128, 1024], f32, tag="s", name="sps2")
            for k in range(8):
                a = PERM[bh * 8 + k]
                c, s4 = a // 4, a % 4
                tp = (96, 0) if s4 == 3 else None
                nc.tensor.matmul(
                    sps[:, colof(k):colof(k) + 128],
                    q2_r[32 * s4:32 * s4 + 16, 4 + c, Jh, :, :, il],
                    q2_r[32 * s4:32 * s4 + 16, c, jh, :, :, il],
                    start=True, stop=True, tile_position=tp)
            return sps

        # per-il tail: normalize + transpose to feature-major + FC + store
        def il_tail(il):
            pa_f = [sb.tile([128, NP], f32r, tag=f"paf{ah}", name=f"paf{ah}",
                            bufs=2) for ah in range(2)]
            for jh in range(2):
                zv = z2_r[jh][:, il, :]
                nc.vector.reciprocal(zv, zv)
                nc.gpsimd.tensor_tensor(
                    pa_r[jh][:, il, :, :], pa_r[jh][:, il, :, :],
                    zv.rearrange("p a -> p a ()").broadcast_to(
                        [128, A_, B_]), op=MUL)
            for ah in range(2):
                ps = psv.tile([128, 512], f32, tag="w", bufs=2, name="pf")
                for jh in range(2):
                    nc.tensor.transpose(
                        ps[:, jh * 128:jh * 128 + 128],
                        pa_tok[jh][:, il * F + ah * 128:
                                   il * F + (ah + 1) * 128],
                        ident[:])
                nc.vector.tensor_copy(pa_f[ah][:, 0:256], ps[:, 0:256])
            for jh in range(2):
                ps = psv.tile([128, 512], f32, tag="w", bufs=2, name="fc")
                for kt in range(2):
                    nc.tensor.matmul(
                        ps[:, 0:F],
                        pa_f[kt][:, jh * 128:(jh + 1) * 128],
                        wc_sb[:, kt * F:(kt + 1) * F],
                        start=(kt == 0), stop=(kt == 1))
                ot = outp.tile([128, F], f32, tag="ot")
                nc.vector.tensor_tensor(ot[:], ps[:, 0:F], bias_sb[:], op=ADD)
                nc.sync.dma_start(out_d[il, jh * 128:(jh + 1) * 128, :], ot[:])

        # task kinds: (Jh keys, jh queries); every kind is a full start/stop
        # AV group; partial sums combine in SBUF (copy then add)
        KINDS = {"q0": (0, 0),   # -> copy to pa0
                 "q1": (1, 0),   # -> add to pa0
                 "c0": (0, 1),   # -> copy to pa1
                 "c1": (1, 1)}   # -> add to pa1

        tps_map = {}

        def process_b(il, bh, kind, sps):
            Jh, jh = KINDS[kind]
            if bh == 0:
                tps_map[(il, kind)] = psv.tile([128, 512], f32, tag="v",
                                               bufs=2, name="tps2")
            tps = tps_map[(il, kind)]
            aex = expp.tile([128, 1024], bf16, tag="aex2", name="aex2")
            nc.scalar.activation(aex[:], sps[:], Exp)
            for k in range(8):
                a = PERM[bh * 8 + k]
                nc.tensor.matmul(
                    tps[:, bh * 136 + k * 17: bh * 136 + k * 17 + 17],
                    aex[:, colof(k):colof(k) + 128],
                    v2t_r[Jh][:, il, a, :], start=True, stop=True)
            if bh == 1:
                jh_dst = jh
                tr = tps_map.pop((il, kind))[:, 0:272] \
                    .rearrange("p (s c) -> p s c", s=16)
                if kind in ("q0", "c0"):
                    nc.vector.tensor_copy(pa_r[jh_dst][:, il, :, :],
                                          tr[:, :, 0:16])
                    nc.vector.tensor_copy(z2_r[jh_dst][:, il, :], tr[:, :, 16])
                else:
                    dst = pa_r[jh_dst][:, il, :, :]
                    nc.vector.tensor_tensor(dst, dst, tr[:, :, 0:16], op=ADD)
                    zdst = z2_r[jh_dst][:, il, :]
                    nc.vector.tensor_tensor(zdst, zdst, tr[:, :, 16], op=ADD)
                    if kind == "c1":
                        il_tail(il)

        # ---- half 0 mixes, then the [J0, j0] quadrant while A2A#2 flies;
        # half 1's loads+mixes interleave into the quadrant pass ----
        load_and_mix(0)
        prev = None
        fill = deque(mix_thunks(1))
        for il in range(NI):
            for bh in range(2):
                sps = emit_scores2(il, bh, 0, 0)
                if prev is not None:
                    process_b(*prev)
                prev = (il, bh, "q0", sps)
                if fill:
                    fill.popleft()()
        while fill:
            fill.popleft()()

        # ---- remaining three quadrants ----
        for il in range(NI):
            for kind in ("q1", "c0", "c1"):
                for bh in range(2):
                    sps = emit_scores2(il, bh, *KINDS[kind])
                    if prev is not None:
                        process_b(*prev)
                    prev = (il, bh, kind, sps)
        process_b(*prev)


# ---------------------------------------------------------------------------
# host side
# ---------------------------------------------------------------------------

def _pad_heads(w, n_in):
    """(n_in, F) with cols f=(a,b) -> (n_in, 4*128): chunk c holds a in
    [4c,4c+4) at col 32*(a%4)+b, cols +16..31 zero."""
    out = np.zeros((n_in, 4 * 128), dtype=np.float32)
    w = w.reshape(n_in, A_, B_)
    for a in range(A_):
        c, s4 = a // 4, a % 4
        out[:, c * 128 + 32 * s4: c * 128 + 32 * s4 + B_] = w[:, a, :]
    return out


def prep_inputs(x, W1, W2, fc1_w, fc1_b, fc2_w, fc2_b):
    """Host-side weight prep + per-core input maps."""
    x = np.asarray(x, dtype=np.float32)
    W1 = np.asarray(W1, dtype=np.float32)
    W2 = np.asarray(W2, dtype=np.float32)
    fc1_w = np.asarray(fc1_w, dtype=np.float32)
    fc1_b = np.asarray(fc1_b, dtype=np.float32)
    fc2_w = np.asarray(fc2_w, dtype=np.float32)
    fc2_b = np.asarray(fc2_b, dtype=np.float32)

    w1q = _pad_heads(W1[0].reshape(D, F), D)
    w1k = _pad_heads(W1[1].reshape(D, F), D)
    w1qk = np.concatenate([w1q, w1k], axis=1).astype(np.float16)
    w1v = W1[2].reshape(D, F).astype(np.float16)

    row_perm = np.array([PERM[pos] * B_ + b for pos in range(A_)
                         for b in range(B_)])
    w2q = _pad_heads(W2[0].reshape(F, F)[row_perm], F)
    w2k = _pad_heads(W2[1].reshape(F, F)[row_perm], F)
    w2qk = np.concatenate([w2q, w2k], axis=1).astype(np.float16)
    w2v = np.ascontiguousarray(W2[2].reshape(F, F)[row_perm]).astype(np.float16)

    wc = np.ascontiguousarray((fc1_w @ fc2_w)[row_perm]).astype(np.float32)
    bc = (fc1_b @ fc2_w + fc2_b).astype(np.float32)
    bc_rep = np.ascontiguousarray(np.broadcast_to(bc, (128, F)))

    in_maps = []
    for s in range(NCORE):
        in_maps.append({
            "x": np.ascontiguousarray(
                x[:, s * PC:(s + 1) * PC, :]).astype(np.float16),
            "w1qk": w1qk, "w1v": w1v,
            "w2qk": w2qk, "w2v": w2v,
            "wc": wc, "bc": bc_rep,
        })
    return in_maps


_CACHE = {}


def kernel(**inputs):
    from concourse.bass_utils import run_bass_kernel_spmd

    in_maps = prep_inputs(**inputs)
    if "nc" not in _CACHE:
        _CACHE["nc"] = build_program("AB", NCORE)
    nc = _CACHE["nc"]
    res = run_bass_kernel_spmd(nc, in_maps, list(range(NCORE)))
    out = np.empty((NF, NP, F), dtype=np.float32)
    for s in range(NCORE):
        out[s * NI:(s + 1) * NI, JPERM, :] = res.results[s]["out"]
    return out


# revision 23
# speedup vs baseline: 1.0570x; 1.0570x over previous
"""Trainium2 Bass kernel for nn_EquivariantAttentionLayer.

Reference computation (N=128 frames, P=256 points, D=128, OUT=256, HEADS=16, HD=16):
  qkv  = einsum('ijd,qdhm->qhmij', x, W1)         # temporal QKV
  s1   = einsum('abij,abIj->aiIj', q, k); a1 = softmax(s1, axis=I)
  t    = einsum('aiIj,abIj->abij', a1, v)
  qkv2 = einsum('hmij,qhmgn->qgnij', t, W2)       # point QKV (mix over both head axes)
  s2   = einsum('abij,abiJ->aijJ', q2, k2); a2 = softmax(s2, axis=J)
  pa   = einsum('aijJ,abiJ->ijab', a2, v2).reshape(N,P,256)
  out  = (pa @ fc1_w + fc1_b) @ fc2_w + fc2_b     # NO nonlinearity -> collapses to one 256x256 matmul

Sharding: phase A is point-sharded (temporal attention is independent per point),
phase B/C are frame-sharded (point attention is independent per frame). Two
half-sized AllToAlls re-shard t from point-shards to frame-shards. Phase A
interleaves QKV prep with attention in 8-point chunks and ships each chunk's
slice as soon as it drains, so AllToAll #1 launches ~40% in. Phase B computes
the [J-half0, j-half0] quadrant of point attention (partial sums + partial Z
in SBUF) while AllToAll #2 is in flight. Engine roles: ACT does only the
softmax exps; DVE does all PSUM evictions (batched wide); Pool (gpsimd, no
PSUM access) does the SBUF-side normalizes and memsets. The FC pair is
collapsed on the host: Wc = fc1_w @ fc2_w ; bc = fc1_b @ fc2_w + fc2_b.
Points are processed in a permuted order (j' = hc*128 + s*16 + jc16); the host
un-permutes the output rows. Heads are processed in PERM order; the host
permutes W2/Wc rows to match.
"""

import numpy as np

# ---- problem dims (hardcoded) ----
NF, NP, D = 128, 256, 128       # frames (i/I), points (j/J), input dim
A_, B_ = 16, 16                 # HD (a/g), HEADS (b/n)
F = A_ * B_                     # 256 features
NCORE = 8
PC = NP // NCORE                # 32 points per core (phase A)
HC = PC // 2                    # 16 points per exchange half
NI = NF // NCORE                # 16 frames per core (phase B)
TOK = NF * PC                   # 4096 tokens per core (both phases)

# Head-processing order: batch bh handles PE row groups {2bh, 2bh+1} so that
# same-PSUM-bank score matmuls are always same-group (HW: cross-group same-bank
# PE writes are fatal).
PERM = [4 * (k // 2) + 2 * bh + (k % 2) for bh in range(2) for k in range(8)]

# Point order as seen by phase B / the raw device output (host un-permutes).
JPERM = np.array([s * PC + hc * HC + jc
                  for hc in range(2) for s in range(NCORE) for jc in range(HC)])


def build_program(phases="AB", n_cores=NCORE, reps=1):
    """Build the SPMD Bass program. phases in {"AB", "A", "B"} (A/B for testing).
    reps>1 repeats the whole body (for wall-clock delta timing)."""
    import concourse.bacc as bacc
    import concourse.mybir as mybir
    import concourse.tile as tile
    from concourse.masks import make_identity

    dt = mybir.dt
    f32 = dt.float32
    f32r = dt.float32r
    f16 = dt.float16

    nc = bacc.Bacc(None, target_bir_lowering=False, num_devices=n_cores)

    if "A" in phases:
        x_d = nc.dram_tensor("x", [NF, PC, D], f16, kind="ExternalInput")
        w1qk_d = nc.dram_tensor("w1qk", [D, 8 * 128], f16, kind="ExternalInput")
        w1v_d = nc.dram_tensor("w1v", [D, F], f16, kind="ExternalInput")
    if "B" in phases:
        w2qk_d = nc.dram_tensor("w2qk", [F, 8 * 128], f16, kind="ExternalInput")
        w2v_d = nc.dram_tensor("w2v", [F, F], f16, kind="ExternalInput")
        wc_d = nc.dram_tensor("wc", [F, F], f32r, kind="ExternalInput")
        bc_d = nc.dram_tensor("bc", [128, F], f32, kind="ExternalInput")
        out_d = nc.dram_tensor("out", [NI, NP, F], f32, kind="ExternalOutput")

    # exchange buffers (per half): tsh[s, f, jc16, il] = t[f, i=s*NI+il, jc]
    kindA = "ExternalOutput" if phases == "A" else None
    kindB = "ExternalInput" if phases == "B" else None
    tsh_ds = tex_ds = None
    if "A" in phases:
        tsh_ds = [[nc.dram_tensor(f"tsh{r}_{h}", [NCORE, F, HC, NI], f16,
                                  **({"kind": kindA} if kindA else {}))
                   for h in range(2)] for r in range(reps)]
    if phases == "AB":
        tex_ds = [[nc.dram_tensor(f"tex{r}_{h}", [NCORE, F, HC, NI], f16)
                   for h in range(2)] for r in range(reps)]
    elif phases == "B":
        tex_ds = [[nc.dram_tensor(f"tex0_{h}", [NCORE, F, HC, NI], f16,
                                  kind="ExternalInput") for h in range(2)]]

    with tile.TileContext(nc) as tc:
        with tc.tile_pool(name="consts", bufs=1) as consts:
            ident = consts.tile([128, 128], f32, tag="ident")
            make_identity(nc, ident[:])

            for r in range(reps):
                if "A" in phases:
                    def do_coll(h, _r=r):
                        if phases != "AB":
                            return
                        nc.gpsimd.collective_compute(
                            "AllToAll", mybir.AluOpType.bypass,
                            replica_groups=[list(range(n_cores))],
                            ins=[tsh_ds[_r][h][:]], outs=[tex_ds[_r][h][:]])
                    _phase_a(nc, tc, tsh_ds[r], do_coll, x_d, w1qk_d, w1v_d,
                             ident, mybir)
                if "B" in phases:
                    _phase_b(nc, tc, out_d, tex_ds[min(r, len(tex_ds) - 1)],
                             w2qk_d, w2v_d, wc_d, bc_d, ident, mybir)

    nc.compile()
    return nc


def _phase_a(nc, tc, tsh_d, do_coll, x_d, w1qk_d, w1v_d, ident, mybir):
    """Temporal QKV + temporal attention for this core's PC points.

    Prep (transpose + QKV mix) and attention interleave in 8-point chunks;
    each chunk's t-slice ships (partial tsh DMA) as soon as it drains."""
    dt = mybir.dt
    f32, f16, bf16 = dt.float32, dt.float16, dt.bfloat16
    Exp = mybir.ActivationFunctionType.Exp
    MUL = mybir.AluOpType.mult

    with tc.tile_pool(name="a_sb", bufs=1) as sb, \
         tc.tile_pool(name="a_ld", bufs=1) as ld, \
         tc.tile_pool(name="a_exp", bufs=3) as expp, \
         tc.tile_pool(name="a_pss", bufs=2, space="PSUM") as pss, \
         tc.tile_pool(name="a_psv", bufs=4, space="PSUM") as psv:

        w1qk_sb = sb.tile([128, 8 * 128], f16, tag="w1qk")
        nc.sync.dma_start(w1qk_sb[:], w1qk_d[:])
        w1v_sb = sb.tile([128, F], f16, tag="w1v")
        nc.sync.dma_start(w1v_sb[:], w1v_d[:])

        # xt_all[d, j*128+i] = x[i, j, d]; loaded transposed via DMA xbar
        xt_all = sb.tile([128, TOK], f16, tag="xt")
        for j in range(PC):
            nc.sync.dma_start_transpose(xt_all[:, j * 128:(j + 1) * 128],
                                        x_d[:, j, :])

        # Q/K, one tile, c-major: chunk c (of 8) holds a in [4c,4c+4), partition
        # 32*(a%4)+b (rows +16..31 zero), free = (c, j, i). c 0-3 = q, 4-7 = k.
        qk = sb.tile([128, 8 * TOK], f16, tag="qk")
        qk_r = qk[:].rearrange("p (c t) -> p c t", c=8)
        # V^T (+ones col): vt[i, (j, a, 17)]; col 16 of each (j,a) block is 1.0
        vt = sb.tile([128, PC * A_ * 17], bf16, tag="vt")
        vt_r = vt[:].rearrange("p (j a c) -> p j a c", j=PC, a=A_, c=17)
        nc.gpsimd.memset(vt_r[:, :, :, 16:17], 1.0)

        Copy = mybir.ActivationFunctionType.Copy
        ei = [0]

        def _mx(nt, cp):
            ps = pss.tile([128, 1024], f32, tag="s", name="mx")
            for ci in range(2):
                nc.tensor.matmul(
                    ps[:, ci * 512:(ci + 1) * 512],
                    w1qk_sb[:, (cp * 2 + ci) * 128:(cp * 2 + ci + 1) * 128],
                    xt_all[:, nt * 512:(nt + 1) * 512],
                    start=True, stop=True)
            dst = qk_r[:, cp * 2:cp * 2 + 2, nt * 512:(nt + 1) * 512]
            srcv = ps[:].rearrange("p (c t) -> p c t", c=2)
            if ei[0] % 4 == 3:
                nc.scalar.activation(dst, srcv, Copy)
            else:
                nc.vector.tensor_copy(dst, srcv)
            ei[0] += 1

        def _vx(pc, g):
            ps = psv.tile([128, 512], f32, tag="w", bufs=2, name="vx")
            for q in range(2):
                j = pc * 8 + g * 2 + q
                nc.tensor.matmul(ps[:, q * F:(q + 1) * F],
                                 xt_all[:, j * 128:(j + 1) * 128],
                                 w1v_sb[:], start=True, stop=True)
            j0 = pc * 8 + g * 2
            nc.vector.tensor_copy(
                vt_r[:, j0:j0 + 2, :, 0:16],
                ps[:].rearrange("p (g a b) -> p g a b", g=2, a=A_))

        def prep_thunks(pc):
            return ([lambda nt=nt, cp=cp: _mx(nt, cp)
                       for nt in (pc * 2, pc * 2 + 1) for cp in range(4)]
                    + [lambda g=g: _vx(pc, g) for g in range(4)])

        def prep_chunk(pc):
            for th in prep_thunks(pc):
                th()

        # temporal attention; tu[i, (j, pos, b)] unnormalized (pos = PERM'd
        # head order); zb[i, (j, pos)] holds Z.
        tu = sb.tile([128, PC * F], f32, tag="tu")
        tu_r = tu[:].rearrange("p (j a b) -> p j a b", j=PC, a=A_)
        zb = sb.tile([128, PC * A_], f32, tag="zb")
        zb_r = zb[:].rearrange("p (j a) -> p j a", j=PC)
        # tab[h]: feature-major t; free = (jh, s, jc16, il) [tsh slice order]
        tab = [sb.tile([128, TOK], f16, tag=f"tab{h}", name=f"tab{h}")
               for h in range(2)]
        colof = lambda k: (k % 2) * 512 + (k // 2) * 128  # bank = row group

        def emit_scores1(j, bh):
            sps = pss.tile([128, 1024], f32, tag="s", name="sps")
            for k in range(8):
                a = PERM[bh * 8 + k]
                c, s4 = a // 4, a % 4
                tp = (96, 0) if s4 == 3 else None
                # S'[I, i]: lhsT=K (b,I), rhs=Q (b,i)
                nc.tensor.matmul(
                    sps[:, colof(k):colof(k) + 128],
                    qk_r[32 * s4:32 * s4 + 16, 4 + c, j * 128:(j + 1) * 128],
                    qk_r[32 * s4:32 * s4 + 16, c, j * 128:(j + 1) * 128],
                    start=True, stop=True, tile_position=tp)
            return sps

        tps_map = {}

        def process_a(j, bh, sps):
            if bh == 0:
                tps_map[j] = psv.tile([128, 512], f32, tag="v", bufs=2,
                                      name="tps")
            tps = tps_map[j]
            aex = expp.tile([128, 1024], bf16, tag="aex", name="aex")
            nc.scalar.activation(aex[:], sps[:], Exp)
            for k in range(8):
                a = PERM[bh * 8 + k]
                # t^T[i, (b,Z)] = A'^T @ [V^T | 1]
                nc.tensor.matmul(tps[:, bh * 136 + k * 17: bh * 136 + k * 17 + 17],
                                 aex[:, colof(k):colof(k) + 128],
                                 vt_r[:, j, a, :], start=True, stop=True)
            if bh == 1:
                tr = tps_map.pop(j)[:, 0:272] \
                    .rearrange("p (s c) -> p s c", s=16)
                nc.vector.tensor_copy(tu_r[:, j, :, :], tr[:, :, 0:16])
                nc.vector.tensor_copy(zb_r[:, j, :], tr[:, :, 16])
                if j % 8 == 7:
                    tail_chunk(j // 8)

        def tail_chunk(ch):
            """Normalize chunk ch's 8 points, transpose to feature-major,
            ship the chunk's tsh slice; A2A after each half's last chunk."""
            jh, jo = ch // 2, ch * 8
            zv = zb_r[:, jo:jo + 8, :]
            nc.vector.reciprocal(zv, zv)
            nc.gpsimd.tensor_tensor(
                tu_r[:, jo:jo + 8, :, :], tu_r[:, jo:jo + 8, :, :],
                zv.rearrange("p j a -> p j a ()").broadcast_to(
                    [128, 8, A_, B_]), op=MUL)
            # transpose to feature-major: per h, 2 quads of 4 points
            tab_v = [tab[h][:, jh * 2048:(jh + 1) * 2048]
                     .rearrange("p (s j i) -> p s j i", s=NCORE, j=HC)
                     for h in range(2)]
            for h in range(2):
                for g in range(2):
                    ps = psv.tile([128, 512], f32, tag="w", bufs=2, name="tt")
                    for q in range(4):
                        j = jo + g * 4 + q
                        nc.tensor.transpose(ps[:, q * 128:(q + 1) * 128],
                                            tu[:, j * F + h * 128:
                                               j * F + (h + 1) * 128],
                                            ident[:])
                    jc0 = (ch % 2) * 8 + g * 4
                    nc.vector.tensor_copy(
                        tab_v[h][:, :, jc0:jc0 + 4, :],
                        ps[:].rearrange("p (j s i) -> p s j i", j=4, s=NCORE))
            jr = slice((ch % 2) * 8, (ch % 2) * 8 + 8)
            for h in range(2):
                nc.sync.dma_start(
                    tsh_d[jh][:, h * 128:(h + 1) * 128, jr, :]
                        .rearrange("s f j i -> f s (j i)"),
                    tab_v[h][:, :, jr, :].rearrange("p s j i -> p s (j i)"))
            if ch % 2 == 1:
                do_coll(jh)

        from collections import deque
        fill = deque()
        prep_chunk(0)
        prev = None
        for ch in range(4):
            if ch + 1 < 4:
                fill.extend(prep_thunks(ch + 1))
            for jc in range(8):
                j = ch * 8 + jc
                for bh in range(2):
                    sps = emit_scores1(j, bh)
                    if prev is not None:
                        process_a(*prev)
                    prev = (j, bh, sps)
                    if fill:
                        fill.popleft()()
        process_a(*prev)


def _phase_b(nc, tc, out_d, tex_d, w2qk_d, w2v_d, wc_d, bc_d, ident, mybir):
    """Point mix + point attention + collapsed FC for this core's NI frames.

    The [J half0, j half0] quadrant of the attention runs right after half 0
    arrives (partial sums + partial Z in SBUF), overlapping AllToAll #2.
    Token order is (il, j') with j' = hc*128 + s*16 + jc16 (host un-permutes)."""
    from collections import deque
    dt = mybir.dt
    f32, f16, bf16, f32r = dt.float32, dt.float16, dt.bfloat16, dt.float32r
    Exp = mybir.ActivationFunctionType.Exp
    MUL = mybir.AluOpType.mult
    ADD = mybir.AluOpType.add

    with tc.tile_pool(name="b_sb", bufs=1) as sb, \
         tc.tile_pool(name="b_exp", bufs=3) as expp, \
         tc.tile_pool(name="b_out", bufs=3) as outp, \
         tc.tile_pool(name="b_pss", bufs=2, space="PSUM") as pss, \
         tc.tile_pool(name="b_psv", bufs=4, space="PSUM") as psv:

        w2v_sb = sb.tile([128, 2 * F], f16, tag="w2v")  # col block kt = rows kt*128..
        nc.sync.dma_start(w2v_sb[:, 0:F], w2v_d[0:128, :])
        nc.sync.dma_start(w2v_sb[:, F:2 * F], w2v_d[128:256, :])
        wc_sb = sb.tile([128, 2 * F], f32r, tag="wc")
        nc.sync.dma_start(wc_sb[:, 0:F], wc_d[0:128, :])
        nc.sync.dma_start(wc_sb[:, F:2 * F], wc_d[128:256, :])
        bias_sb = sb.tile([128, F], f32, tag="bias")
        nc.sync.dma_start(bias_sb[:], bc_d[:])
        w2qk_sb = [sb.tile([128, 1024], f16, tag=f"w2qk{kt}",
                           name=f"w2qk{kt}") for kt in range(2)]
        for kt in range(2):
            nc.sync.dma_start(w2qk_sb[kt][:], w2qk_d[kt * 128:(kt + 1) * 128, :])

        # q2/k2, one tile, c-major; free = (c, hc, s, jc16, il)
        q2k2 = sb.tile([128, 8 * TOK], f16, tag="q2k2")
        q2_r = q2k2[:].rearrange("p (c hc s j i) -> p c hc s j i",
                                 c=8, hc=2, s=NCORE, j=HC)
        # v2t[hc][j'_loc, (il, a, 17)]
        v2t = [sb.tile([128, NI * A_ * 17], bf16, tag=f"v2t{h}", name=f"v2t{h}")
               for h in range(2)]
        v2t_r = [v2t[h][:].rearrange("p (i a c) -> p i a c", i=NI, a=A_)
                 for h in range(2)]
        for h in range(2):
            nc.gpsimd.memset(v2t_r[h][:, :, :, 16:17], 1.0)
        # t2[h][f_local, (hc, s, jc16, il)]
        t2 = [sb.tile([128, TOK], f16, tag=f"t2_{h}", name=f"t2_{h}")
              for h in range(2)]
        t2_r = [t2[h][:].rearrange("p (hc s j i) -> p hc s j i",
                                   hc=2, s=NCORE, j=HC) for h in range(2)]

        def _t2load(hc):
            for h in range(2):
                for s in range(NCORE):
                    nc.sync.dma_start(
                        t2_r[h][:, hc, s, :, :],
                        tex_d[hc][s, h * 128:(h + 1) * 128, :, :])

        def _mx2(hc, nt, cp):
            ps = pss.tile([128, 1024], f32, tag="s", name="mx2")
            for ci in range(2):
                c = cp * 2 + ci
                for kt in range(2):
                    nc.tensor.matmul(
                        ps[:, ci * 512:(ci + 1) * 512],
                        w2qk_sb[kt][:, c * 128:(c + 1) * 128],
                        t2[kt][:, hc * 2048 + nt * 512:
                               hc * 2048 + (nt + 1) * 512],
                        start=(kt == 0), stop=(kt == 1))
            nc.vector.tensor_copy(
                q2_r[:, cp * 2:cp * 2 + 2, hc, nt * 2:(nt + 1) * 2, :, :]
                    .rearrange("p c s j i -> p c (s j i)"),
                ps[:].rearrange("p (c t) -> p c t", c=2))

        def _v2x(hc, ip):
            ps = psv.tile([128, 512], f32, tag="w", bufs=2, name="v2x")
            for q in range(2):
                il = ip * 2 + q
                for kt in range(2):
                    nc.tensor.matmul(
                        ps[:, q * F:(q + 1) * F],
                        t2_r[kt][:, hc, :, :, il],
                        w2v_sb[:, kt * F:(kt + 1) * F],
                        start=(kt == 0), stop=(kt == 1))
            nc.vector.tensor_copy(
                v2t_r[hc][:, ip * 2:ip * 2 + 2, :, 0:16],
                ps[:].rearrange("p (g a b) -> p g a b", g=2, a=A_))

        def mix_thunks(hc):
            return ([lambda: _t2load(hc)]
                    + [lambda nt=nt, cp=cp: _mx2(hc, nt, cp)
                       for nt in range(4) for cp in range(4)]
                    + [lambda ip=ip: _v2x(hc, ip) for ip in range(NI // 2)])

        def load_and_mix(hc):
            for th in mix_thunks(hc):
                th()

        # point attention: pa_tok[jh][j'_loc, (il, pos, b)] unnorm; z2 = Z
        pa_tok = [sb.tile([128, NI * F], f32, tag=f"pat{jh}",
                          name=f"pat{jh}") for jh in range(2)]
        pa_r = [pa_tok[jh][:].rearrange("p (i a b) -> p i a b", i=NI, a=A_)
                for jh in range(2)]
        z2 = [sb.tile([128, NI * A_], f32, tag=f"z2_{jh}", name=f"z2_{jh}")
              for jh in range(2)]
        z2_r = [z2[jh][:].rearrange("p (i a) -> p i a", i=NI)
                for jh in range(2)]
        colof = lambda k: (k % 2) * 512 + (k // 2) * 128

        def emit_scores2(il, bh, Jh, jh):
            # 8 heads x [J' = half Jh (128 keys), j = half jh (128 queries)]
            sps = pss.tile([128, 1024], f32, tag="s", name="sps2")
            for k in range(8):
                a = PERM[bh * 8 + k]
                c, s4 = a // 4, a % 4
                tp = (96, 0) if s4 == 3 else None
                nc.tensor.matmul(
                    sps[:, colof(k):colof(k) + 128],
                    q2_r[32 * s4:32 * s4 + 16, 4 + c, Jh, :, :, il],
                    q2_r[32 * s4:32 * s4 + 16, c, jh, :, :, il],
                    start=True, stop=True, tile_position=tp)
            return sps

        # per-il tail: normalize + transpose to feature-major + FC + store
        def il_tail(il):
            pa_f = [sb.tile([128, NP], f32r, tag=f"paf{ah}", name=f"paf{ah}",
                            bufs=2) for ah in range(2)]
            for jh in range(2):
                zv = z2_r[jh][:, il, :]
                nc.vector.reciprocal(zv, zv)
                nc.gpsimd.tensor_tensor(
                    pa_r[jh][:, il, :, :], pa_r[jh][:, il, :, :],
                    zv.rearrange("p a -> p a ()").broadcast_to(
                        [128, A_, B_]), op=MUL)
            for ah in range(2):
                ps = psv.tile([128, 512], f32, tag="w", bufs=2, name="pf")
                for jh in range(2):
                    nc.tensor.transpose(
                        ps[:, jh * 128:jh * 128 + 128],
                        pa_tok[jh][:, il * F + ah * 128:
                                   il * F + (ah + 1) * 128],
                        ident[:])
                nc.vector.tensor_copy(pa_f[ah][:, 0:256], ps[:, 0:256])
            for jh in range(2):
                ps = psv.tile([128, 512], f32, tag="w", bufs=2, name="fc")
                for kt in range(2):
                    nc.tensor.matmul(
                        ps[:, 0:F],
                        pa_f[kt][:, jh * 128:(jh + 1) * 128],
                        wc_sb[:, kt * F:(kt + 1) * F],
                        start=(kt == 0), stop=(kt == 1))
                ot = outp.tile([128, F], f32, tag="ot")
                nc.vector.tensor_tensor(ot[:], ps[:, 0:F], bias_sb[:], op=ADD)
                nc.sync.dma_start(out_d[il, jh * 128:(jh + 1) * 128, :], ot[:])

        # task kinds: (Jh keys, jh queries); every kind is a full start/stop
        # AV group; partial sums combine in SBUF (copy then add)
        KINDS = {"q0": (0, 0),   # -> copy to pa0
                 "q1": (1, 0),   # -> add to pa0
                 "c0": (0, 1),   # -> copy to pa1
                 "c1": (1, 1)}   # -> add to pa1

        tps_map = {}

        def process_b(il, bh, kind, sps):
            Jh, jh = KINDS[kind]
            if bh == 0:
                tps_map[(il, kind)] = psv.tile([128, 512], f32, tag="v",
                                               bufs=2, name="tps2")
            tps = tps_map[(il, kind)]
            aex = expp.tile([128, 1024], bf16, tag="aex2", name="aex2")
            nc.scalar.activation(aex[:], sps[:], Exp)
            for k in range(8):
                a = PERM[bh * 8 + k]
                nc.tensor.matmul(
                    tps[:, bh * 136 + k * 17: bh * 136 + k * 17 + 17],
                    aex[:, colof(k):colof(k) + 128],
                    v2t_r[Jh][:, il, a, :], start=True, stop=True)
            if bh == 1:
                jh_dst = jh
                tr = tps_map.pop((il, kind))[:, 0:272] \
                    .rearrange("p (s c) -> p s c", s=16)
                if kind in ("q0", "c0"):
                    nc.vector.tensor_copy(pa_r[jh_dst][:, il, :, :],
                                          tr[:, :, 0:16])
                    nc.vector.tensor_copy(z2_r[jh_dst][:, il, :], tr[:, :, 16])
                else:
                    dst = pa_r[jh_dst][:, il, :, :]
                    nc.vector.tensor_tensor(dst, dst, tr[:, :, 0:16], op=ADD)
                    zdst = z2_r[jh_dst][:, il, :]
                    nc.vector.tensor_tensor(zdst, zdst, tr[:, :, 16], op=ADD)
                    if kind == "c1":
                        il_tail(il)

        # ---- half 0 mixes, then the [J0, j0] quadrant while A2A#2 flies.
        # Half 1's t2 loads issue early (SP blocks harmlessly); its mixes
        # must NOT interleave into the quadrant: their matmuls wait on
        # A2A#2 and would stall the in-order PE queue. ----
        load_and_mix(0)
        _t2load(1)
        prev = None
        for il in range(NI):
            for bh in range(2):
                sps = emit_scores2(il, bh, 0, 0)
                if prev is not None:
                    process_b(*prev)
                prev = (il, bh, "q0", sps)
        for th in mix_thunks(1)[1:]:
            th()

        # ---- remaining three quadrants ----
        for il in range(NI):
            for kind in ("q1", "c0", "c1"):
                for bh in range(2):
                    sps = emit_scores2(il, bh, *KINDS[kind])
                    if prev is not None:
                        process_b(*prev)
                    prev = (il, bh, kind, sps)
        process_b(*prev)


# ---------------------------------------------------------------------------
# host side
# ---------------------------------------------------------------------------

def _pad_heads(w, n_in):
    """(n_in, F) with cols f=(a,b) -> (n_in, 4*128): chunk c holds a in
    [4c,4c+4) at col 32*(a%4)+b, cols +16..31 zero."""
    out = np.zeros((n_in, 4 * 128), dtype=np.float32)
    w = w.reshape(n_in, A_, B_)
    for a in range(A_):
        c, s4 = a // 4, a % 4
        out[:, c * 128 + 32 * s4: c * 128 + 32 * s4 + B_] = w[:, a, :]
    return out


def prep_inputs(x, W1, W2, fc1_w, fc1_b, fc2_w, fc2_b):
    """Host-side weight prep + per-core input maps."""
    x = np.asarray(x, dtype=np.float32)
    W1 = np.asarray(W1, dtype=np.float32)
    W2 = np.asarray(W2, dtype=np.float32)
    fc1_w = np.asarray(fc1_w, dtype=np.float32)
    fc1_b = np.asarray(fc1_b, dtype=np.float32)
    fc2_w = np.asarray(fc2_w, dtype=np.float32)
    fc2_b = np.asarray(fc2_b, dtype=np.float32)

    w1q = _pad_heads(W1[0].reshape(D, F), D)
    w1k = _pad_heads(W1[1].reshape(D, F), D)
    w1qk = np.concatenate([w1q, w1k], axis=1).astype(np.float16)
    w1v = W1[2].reshape(D, F).astype(np.float16)

    row_perm = np.array([PERM[pos] * B_ + b for pos in range(A_)
                         for b in range(B_)])
    w2q = _pad_heads(W2[0].reshape(F, F)[row_perm], F)
    w2k = _pad_heads(W2[1].reshape(F, F)[row_perm], F)
    w2qk = np.concatenate([w2q, w2k], axis=1).astype(np.float16)
    w2v = np.ascontiguousarray(W2[2].reshape(F, F)[row_perm]).astype(np.float16)

    wc = np.ascontiguousarray((fc1_w @ fc2_w)[row_perm]).astype(np.float32)
    bc = (fc1_b @ fc2_w + fc2_b).astype(np.float32)
    bc_rep = np.ascontiguousarray(np.broadcast_to(bc, (128, F)))

    in_maps = []
    for s in range(NCORE):
        in_maps.append({
            "x": np.ascontiguousarray(
                x[:, s * PC:(s + 1) * PC, :]).astype(np.float16),
            "w1qk": w1qk, "w1v": w1v,
            "w2qk": w2qk, "w2v": w2v,
            "wc": wc, "bc": bc_rep,
        })
    return in_maps


_CACHE = {}


def kernel(**inputs):
    from concourse.bass_utils import run_bass_kernel_spmd

    in_maps = prep_inputs(**inputs)
    if "nc" not in _CACHE:
        _CACHE["nc"] = build_program("AB", NCORE)
    nc = _CACHE["nc"]
    res = run_bass_kernel_spmd(nc, in_maps, list(range(NCORE)))
    out = np.empty((NF, NP, F), dtype=np.float32)
    for s in range(NCORE):
        out[s * NI:(s + 1) * NI, JPERM, :] = res.results[s]["out"]
    return out


# revision 47
# speedup vs baseline: 3.8204x; 3.6144x over previous
"""Trainium2 Bass kernel for nn_EquivariantAttentionLayer.

Reference computation (N=128 frames, P=256 points, D=128, OUT=256, HEADS=16, HD=16):
  qkv  = einsum('ijd,qdhm->qhmij', x, W1)         # temporal QKV
  s1   = einsum('abij,abIj->aiIj', q, k); a1 = softmax(s1, axis=I)
  t    = einsum('aiIj,abIj->abij', a1, v)
  qkv2 = einsum('hmij,qhmgn->qgnij', t, W2)       # point QKV (mix over both head axes)
  s2   = einsum('abij,abiJ->aijJ', q2, k2); a2 = softmax(s2, axis=J)
  pa   = einsum('aijJ,abiJ->ijab', a2, v2).reshape(N,P,256)
  out  = (pa @ fc1_w + fc1_b) @ fc2_w + fc2_b     # NO nonlinearity -> collapses to one 256x256 matmul

Sharding: phase A is point-sharded (temporal attention is independent per point),
phase B/C are frame-sharded (point attention is independent per frame). Two
half-sized AllToAlls re-shard t from point-shards to frame-shards. Phase A
interleaves QKV prep with attention in 8-point chunks and ships each chunk's
slice as soon as it drains, so AllToAll #1 launches ~40% in. Phase B computes
the [J-half0, j-half0] quadrant of point attention (partial sums + partial Z
in SBUF) while AllToAll #2 is in flight. Engine roles: ACT does only the
softmax exps; DVE does all PSUM evictions (batched wide); Pool (gpsimd, no
PSUM access) does the SBUF-side normalizes and memsets. The FC pair is
collapsed on the host: Wc = fc1_w @ fc2_w ; bc = fc1_b @ fc2_w + fc2_b.
Points are processed in a permuted order (j' = hc*128 + s*16 + jc16); the host
un-permutes the output rows. Heads are processed in PERM order; the host
permutes W2/Wc rows to match.
"""

import numpy as np

# ---- problem dims (hardcoded) ----
NF, NP, D = 128, 256, 128       # frames (i/I), points (j/J), input dim
A_, B_ = 16, 16                 # HD (a/g), HEADS (b/n)
F = A_ * B_                     # 256 features
NCORE = 8
PC = NP // NCORE                # 32 points per core (phase A)
HC = PC // 2                    # 16 points per exchange half
NI = NF // NCORE                # 16 frames per core (phase B)
TOK = NF * PC                   # 4096 tokens per core (both phases)

# Head-processing order: batch bh handles PE row groups {2bh, 2bh+1} so that
# same-PSUM-bank score matmuls are always same-group (HW: cross-group same-bank
# PE writes are fatal).
PERM = [4 * (k // 2) + 2 * bh + (k % 2) for bh in range(2) for k in range(8)]

# Point order as seen by phase B / the raw device output (host un-permutes).
JPERM = np.array([s * PC + hc * HC + jc
                  for hc in range(2) for s in range(NCORE) for jc in range(HC)])


def build_program(phases="AB", n_cores=NCORE, reps=1):
    """Build the SPMD Bass program. phases in {"AB", "A", "B"} (A/B for testing).
    reps>1 repeats the whole body (for wall-clock delta timing)."""
    import concourse.bacc as bacc
    import concourse.mybir as mybir
    import concourse.tile as tile
    from concourse.masks import make_identity

    dt = mybir.dt
    f32 = dt.float32
    f32r = dt.float32r
    f16 = dt.float16

    nc = bacc.Bacc(None, target_bir_lowering=False, num_devices=n_cores)

    if "A" in phases:
        x_d = nc.dram_tensor("x", [NF, PC, D], f16, kind="ExternalInput")
        w1qk_d = nc.dram_tensor("w1qk", [D, 8 * 128], f16, kind="ExternalInput")
        w1v_d = nc.dram_tensor("w1v", [D, F], f16, kind="ExternalInput")
    if "B" in phases:
        w2qk_d = nc.dram_tensor("w2qk", [F, 8 * 128], f16, kind="ExternalInput")
        w2v_d = nc.dram_tensor("w2v", [F, F], f16, kind="ExternalInput")
        wc_d = nc.dram_tensor("wc", [F, F], f32r, kind="ExternalInput")
        bc_d = nc.dram_tensor("bc", [128, F], f32, kind="ExternalInput")
        out_d = nc.dram_tensor("out", [NI, NP, F], f32, kind="ExternalOutput")

    # exchange buffers: tsh[s, f, jc, il] = t[f, i=s*NI+il, jc]. Half 0
    # ships as two 8-point quarters (collectives 0,1) so the first AllToAll
    # launches right after chunk 0; half 1 ships whole (collective 2).
    SHAPES = [[NCORE, F, 8, NI], [NCORE, F, 8, NI], [NCORE, F, HC, NI]]
    kindA = "ExternalOutput" if phases == "A" else None
    kindB = "ExternalInput" if phases == "B" else None
    tsh_ds = tex_ds = None
    if "A" in phases:
        tsh_ds = [[nc.dram_tensor(f"tsh{r}_{q}", SHAPES[q], f16,
                                  **({"kind": kindA} if kindA else {}))
                   for q in range(3)] for r in range(reps)]
    if phases == "AB":
        tex_ds = [[nc.dram_tensor(f"tex{r}_{q}", SHAPES[q], f16)
                   for q in range(3)] for r in range(reps)]
    elif phases == "B":
        tex_ds = [[nc.dram_tensor(f"tex0_{q}", SHAPES[q], f16,
                                  kind="ExternalInput") for q in range(3)]]

    with tile.TileContext(nc) as tc:
        with tc.tile_pool(name="consts", bufs=1) as consts:
            ident = consts.tile([128, 128], f32, tag="ident")
            make_identity(nc, ident[:])

            for r in range(reps):
                if "A" in phases:
                    def do_coll(h, _r=r):
                        if phases != "AB":
                            return
                        nc.gpsimd.collective_compute(
                            "AllToAll", mybir.AluOpType.bypass,
                            replica_groups=[list(range(n_cores))],
                            ins=[tsh_ds[_r][h][:]], outs=[tex_ds[_r][h][:]])
                    _phase_a(nc, tc, tsh_ds[r], do_coll, x_d, w1qk_d, w1v_d,
                             ident, mybir)
                if "B" in phases:
                    _phase_b(nc, tc, out_d, tex_ds[min(r, len(tex_ds) - 1)],
                             w2qk_d, w2v_d, wc_d, bc_d, ident, mybir)

    nc.compile()
    return nc


def _phase_a(nc, tc, tsh_d, do_coll, x_d, w1qk_d, w1v_d, ident, mybir):
    """Temporal QKV + temporal attention for this core's PC points.

    Prep (transpose + QKV mix) and attention interleave in 8-point chunks;
    each chunk's t-slice ships (partial tsh DMA) as soon as it drains."""
    dt = mybir.dt
    f32, f16, bf16 = dt.float32, dt.float16, dt.bfloat16
    Exp = mybir.ActivationFunctionType.Exp
    MUL = mybir.AluOpType.mult

    with tc.tile_pool(name="a_sb", bufs=1) as sb, \
         tc.tile_pool(name="a_exp", bufs=4) as expp, \
         tc.tile_pool(name="a_pss", bufs=2, space="PSUM") as pss, \
         tc.tile_pool(name="a_psv", bufs=4, space="PSUM") as psv:

        w1qk_sb = sb.tile([128, 8 * 128], f16, tag="w1qk")
        nc.sync.dma_start(w1qk_sb[:], w1qk_d[:])
        w1v_sb = sb.tile([128, F], f16, tag="w1v")
        nc.sync.dma_start(w1v_sb[:], w1v_d[:])

        # xt_all[d, j*128+i] = x[i, j, d]; loaded transposed via DMA xbar
        xt_all = sb.tile([128, TOK], f16, tag="xt")
        for j in range(PC):
            nc.sync.dma_start_transpose(xt_all[:, j * 128:(j + 1) * 128],
                                        x_d[:, j, :])

        # Q/K, one tile, c-major: chunk c (of 8) holds a in [4c,4c+4), partition
        # 32*(a%4)+b (rows +16..31 zero), free = (c, j, i). c 0-3 = q, 4-7 = k.
        qk = sb.tile([128, 8 * TOK], f16, tag="qk")
        qk_r = qk[:].rearrange("p (c t) -> p c t", c=8)
        # V^T (+ones col): vt[i, (j, a, 17)]; col 16 of each (j,a) block is 1.0
        vt = sb.tile([128, PC * A_ * 17], bf16, tag="vt")
        vt_r = vt[:].rearrange("p (j a c) -> p j a c", j=PC, a=A_, c=17)
        nc.gpsimd.memset(vt_r[:, :, :, 16:17], 1.0)

        Copy = mybir.ActivationFunctionType.Copy
        ei = [0]

        def _mx(nt, cp):
            ps = pss.tile([128, 1024], f32, tag="s", name="mx")
            for ci in range(2):
                nc.tensor.matmul(
                    ps[:, ci * 512:(ci + 1) * 512],
                    w1qk_sb[:, (cp * 2 + ci) * 128:(cp * 2 + ci + 1) * 128],
                    xt_all[:, nt * 512:(nt + 1) * 512],
                    start=True, stop=True)
            dst = qk_r[:, cp * 2:cp * 2 + 2, nt * 512:(nt + 1) * 512]
            srcv = ps[:].rearrange("p (c t) -> p c t", c=2)
            # chunk 0's prep runs before any exp exists: ACT is idle, use it
            if ei[0] % 2 == 1:
                nc.scalar.activation(dst, srcv, Copy)
            else:
                nc.vector.tensor_copy(dst, srcv)
            ei[0] += 1

        def _vx(pc, g):
            ps = psv.tile([128, 512], f32, tag="w", bufs=2, name="vx")
            for q in range(2):
                j = pc * 8 + g * 2 + q
                nc.tensor.matmul(ps[:, q * F:(q + 1) * F],
                                 xt_all[:, j * 128:(j + 1) * 128],
                                 w1v_sb[:], start=True, stop=True)
            j0 = pc * 8 + g * 2
            nc.vector.tensor_copy(
                vt_r[:, j0:j0 + 2, :, 0:16],
                ps[:].rearrange("p (g a b) -> p g a b", g=2, a=A_))

        def prep_thunks(pc):
            return ([lambda nt=nt, cp=cp: _mx(nt, cp)
                       for nt in (pc * 2, pc * 2 + 1) for cp in range(4)]
                    + [lambda g=g: _vx(pc, g) for g in range(4)])

        def prep_chunk(pc):
            for th in prep_thunks(pc):
                th()

        # temporal attention; tu[i, (j, pos, b)] unnormalized (pos = PERM'd
        # head order); zb[i, (j, pos)] holds Z.
        tu = sb.tile([128, PC * F], f32, tag="tu")
        tu_r = tu[:].rearrange("p (j a b) -> p j a b", j=PC, a=A_)
        zb = sb.tile([128, PC * A_], f32, tag="zb")
        zb_r = zb[:].rearrange("p (j a) -> p j a", j=PC)
        # tab[h]: feature-major t; free = (jh, s, jc16, il) [tsh slice order]
        tab = [sb.tile([128, TOK], f16, tag=f"tab{h}", name=f"tab{h}")
               for h in range(2)]
        colof = lambda k: (k % 2) * 512 + (k // 2) * 128  # bank = row group

        def emit_scores1(j, bh):
            sps = pss.tile([128, 1024], f32, tag="s", name="sps")
            for k in range(8):
                a = PERM[bh * 8 + k]
                c, s4 = a // 4, a % 4
                tp = (96, 0) if s4 == 3 else None
                # S'[I, i]: lhsT=K (b,I), rhs=Q (b,i)
                nc.tensor.matmul(
                    sps[:, colof(k):colof(k) + 128],
                    qk_r[32 * s4:32 * s4 + 16, 4 + c, j * 128:(j + 1) * 128],
                    qk_r[32 * s4:32 * s4 + 16, c, j * 128:(j + 1) * 128],
                    start=True, stop=True, tile_position=tp)
            return sps

        tps_map = {}

        def process_a(j, bh, sps):
            if bh == 0:
                tps_map[j] = psv.tile([128, 512], f32, tag="v", bufs=2,
                                      name="tps")
            tps = tps_map[j]
            aex = expp.tile([128, 1024], bf16, tag="aex", name="aex")
            nc.scalar.activation(aex[:], sps[:], Exp)
            for k in range(8):
                a = PERM[bh * 8 + k]
                # t^T[i, (b,Z)] = A'^T @ [V^T | 1]
                nc.tensor.matmul(tps[:, bh * 136 + k * 17: bh * 136 + k * 17 + 17],
                                 aex[:, colof(k):colof(k) + 128],
                                 vt_r[:, j, a, :], start=True, stop=True)
            if bh == 1:
                tr = tps_map.pop(j)[:, 0:272] \
                    .rearrange("p (s c) -> p s c", s=16)
                nc.vector.tensor_copy(tu_r[:, j, :, :], tr[:, :, 0:16])
                nc.vector.tensor_copy(zb_r[:, j, :], tr[:, :, 16])
                if j % 2 == 1:
                    # normalize the drained pair now: keeps it off the
                    # ship-critical tail. Chunk-final pair goes on DVE
                    # (low latency); others on Pool (offload).
                    zv = zb_r[:, j - 1:j + 1, :]
                    nc.vector.reciprocal(zv, zv)
                    zbc = zv.rearrange("p j a -> p j a ()").broadcast_to(
                        [128, 2, A_, B_])
                    dat = tu_r[:, j - 1:j + 1, :, :]
                    # DVE, not Pool: the collectives issue from Pool's
                    # in-order queue and must not sit behind normalizes
                    nc.vector.tensor_tensor(dat, dat, zbc, op=MUL)
                    if j % 8 == 7:
                        tail_chunk(j // 8)

        def tail_chunk(ch):
            """Normalize chunk ch's 8 points, transpose to feature-major,
            ship the chunk's tsh slice; A2A after each half's last chunk."""
            jh, jo = ch // 2, ch * 8
            # transpose to feature-major: per h, 2 quads of 4 points
            tab_v = [tab[h][:, jh * 2048:(jh + 1) * 2048]
                     .rearrange("p (s j i) -> p s j i", s=NCORE, j=HC)
                     for h in range(2)]
            for h in range(2):
                for g in range(2):
                    ps = psv.tile([128, 512], f32, tag="w", bufs=2, name="tt")
                    for q in range(4):
                        j = jo + g * 4 + q
                        nc.tensor.transpose(ps[:, q * 128:(q + 1) * 128],
                                            tu[:, j * F + h * 128:
                                               j * F + (h + 1) * 128],
                                            ident[:])
                    jc0 = (ch % 2) * 8 + g * 4
                    nc.vector.tensor_copy(
                        tab_v[h][:, :, jc0:jc0 + 4, :],
                        ps[:].rearrange("p (j s i) -> p s j i", j=4, s=NCORE))
            if ch < 2:   # half-0 quarters ship (and fly) individually
                for h in range(2):
                    nc.sync.dma_start(
                        tsh_d[ch][:, h * 128:(h + 1) * 128, :, :]
                            .rearrange("s f j i -> f s (j i)"),
                        tab_v[h][:, :, (ch % 2) * 8:(ch % 2) * 8 + 8, :]
                            .rearrange("p s j i -> p s (j i)"))
                do_coll(ch)
            else:
                jr = slice((ch % 2) * 8, (ch % 2) * 8 + 8)
                for h in range(2):
                    nc.sync.dma_start(
                        tsh_d[2][:, h * 128:(h + 1) * 128, jr, :]
                            .rearrange("s f j i -> f s (j i)"),
                        tab_v[h][:, :, jr, :].rearrange("p s j i -> p s (j i)"))
                if ch == 3:
                    do_coll(2)

        from collections import deque
        fill = deque()
        prep_chunk(0)
        prev = None
        for ch in range(4):
            if ch + 1 < 4:
                fill.extend(prep_thunks(ch + 1))
            for jc in range(8):
                j = ch * 8 + jc
                for bh in range(2):
                    sps = emit_scores1(j, bh)
                    if prev is not None:
                        process_a(*prev)
                    prev = (j, bh, sps)
                    if fill:
                        fill.popleft()()
        process_a(*prev)


def _phase_b(nc, tc, out_d, tex_d, w2qk_d, w2v_d, wc_d, bc_d, ident, mybir):
    """Point mix + point attention + collapsed FC for this core's NI frames.

    The [J half0, j half0] quadrant of the attention runs right after half 0
    arrives (partial sums + partial Z in SBUF), overlapping AllToAll #2.
    Token order is (il, j') with j' = hc*128 + s*16 + jc16 (host un-permutes)."""
    from collections import deque
    dt = mybir.dt
    f32, f16, bf16, f32r = dt.float32, dt.float16, dt.bfloat16, dt.float32r
    Exp = mybir.ActivationFunctionType.Exp
    MUL = mybir.AluOpType.mult
    ADD = mybir.AluOpType.add

    with tc.tile_pool(name="b_sb", bufs=1) as sb, \
         tc.tile_pool(name="b_exp", bufs=4) as expp, \
         tc.tile_pool(name="b_out", bufs=3) as outp, \
         tc.tile_pool(name="b_pss", bufs=2, space="PSUM") as pss, \
         tc.tile_pool(name="b_psv", bufs=4, space="PSUM") as psv:

        w2v_sb = sb.tile([128, 2 * F], f16, tag="w2v")  # col block kt = rows kt*128..
        nc.sync.dma_start(w2v_sb[:, 0:F], w2v_d[0:128, :])
        nc.sync.dma_start(w2v_sb[:, F:2 * F], w2v_d[128:256, :])
        wc_sb = sb.tile([128, 2 * F], f32r, tag="wc")
        nc.sync.dma_start(wc_sb[:, 0:F], wc_d[0:128, :])
        nc.sync.dma_start(wc_sb[:, F:2 * F], wc_d[128:256, :])
        bias_sb = sb.tile([128, F], f32, tag="bias")
        nc.sync.dma_start(bias_sb[:], bc_d[:])
        w2qk_sb = [sb.tile([128, 1024], f16, tag=f"w2qk{kt}",
                           name=f"w2qk{kt}") for kt in range(2)]
        for kt in range(2):
            nc.sync.dma_start(w2qk_sb[kt][:], w2qk_d[kt * 128:(kt + 1) * 128, :])

        # q2/k2, one tile, c-major; free = (c, hc, s, jc16, il)
        q2k2 = sb.tile([128, 8 * TOK], f16, tag="q2k2")
        q2_r = q2k2[:].rearrange("p (c hc s j i) -> p c hc s j i",
                                 c=8, hc=2, s=NCORE, j=HC)
        # v2t[hc][j'_loc, (il, a, 17)]
        v2t = [sb.tile([128, NI * A_ * 17], bf16, tag=f"v2t{h}", name=f"v2t{h}")
               for h in range(2)]
        v2t_r = [v2t[h][:].rearrange("p (i a c) -> p i a c", i=NI, a=A_)
                 for h in range(2)]
        for h in range(2):
            nc.gpsimd.memset(v2t_r[h][:, :, :, 16:17], 1.0)
        # t2[h][f_local, (hc, s, jc16, il)]
        t2 = [sb.tile([128, TOK], f16, tag=f"t2_{h}", name=f"t2_{h}")
              for h in range(2)]
        t2_r = [t2[h][:].rearrange("p (hc s j i) -> p hc s j i",
                                   hc=2, s=NCORE, j=HC) for h in range(2)]

        def _t2load(hc):
            # one DMA per (feature-block, exchange tensor): dest (s, jc, il)
            # contiguous; src balances as partition + (s, jc*il)
            for h in range(2):
                if hc == 0:
                    for q in range(2):
                        nc.sync.dma_start(
                            t2_r[h][:, 0, :, q * 8:(q + 1) * 8, :],
                            tex_d[q][:, h * 128:(h + 1) * 128, :, :]
                                .rearrange("s f j i -> f s (j i)"))
                else:
                    nc.sync.dma_start(
                        t2_r[h][:, 1, :, :, :],
                        tex_d[2][:, h * 128:(h + 1) * 128, :, :]
                            .rearrange("s f j i -> f s (j i)"))

        Copy = mybir.ActivationFunctionType.Copy
        mei = [0]

        def _mx2(hc, nt, cp):
            ps = pss.tile([128, 1024], f32, tag="s", name="mx2")
            for ci in range(2):
                c = cp * 2 + ci
                for kt in range(2):
                    nc.tensor.matmul(
                        ps[:, ci * 512:(ci + 1) * 512],
                        w2qk_sb[kt][:, c * 128:(c + 1) * 128],
                        t2[kt][:, hc * 2048 + nt * 512:
                               hc * 2048 + (nt + 1) * 512],
                        start=(kt == 0), stop=(kt == 1))
            # ACT is exp-starved while mixes gate the next pass: split the
            # evictions between DVE and ACT so neither serializes the chain
            dst = q2_r[:, cp * 2:cp * 2 + 2, hc, nt * 2:(nt + 1) * 2, :, :] \
                .rearrange("p c s j i -> p c (s j i)")
            srcv = ps[:].rearrange("p (c t) -> p c t", c=2)
            if hc == 1 and mei[0] % 2 == 1:
                nc.scalar.activation(dst, srcv, Copy)
            else:
                nc.vector.tensor_copy(dst, srcv)
            mei[0] += 1

        def _v2x(hc, ip):
            ps = psv.tile([128, 512], f32, tag="w", bufs=2, name="v2x")
            for q in range(2):
                il = ip * 2 + q
                for kt in range(2):
                    nc.tensor.matmul(
                        ps[:, q * F:(q + 1) * F],
                        t2_r[kt][:, hc, :, :, il],
                        w2v_sb[:, kt * F:(kt + 1) * F],
                        start=(kt == 0), stop=(kt == 1))
            nc.vector.tensor_copy(
                v2t_r[hc][:, ip * 2:ip * 2 + 2, :, 0:16],
                ps[:].rearrange("p (g a b) -> p g a b", g=2, a=A_))

        def mix_thunks(hc):
            return ([lambda: _t2load(hc)]
                    + [lambda nt=nt, cp=cp: _mx2(hc, nt, cp)
                       for nt in range(4) for cp in range(4)]
                    + [lambda ip=ip: _v2x(hc, ip) for ip in range(NI // 2)])

        def load_and_mix(hc):
            for th in mix_thunks(hc):
                th()

        # point attention: pa_tok[jh][j'_loc, (il, pos, b)] unnorm; z2 = Z
        pa_tok = [sb.tile([128, NI * F], f32, tag=f"pat{jh}",
                          name=f"pat{jh}") for jh in range(2)]
        pa_r = [pa_tok[jh][:].rearrange("p (i a b) -> p i a b", i=NI, a=A_)
                for jh in range(2)]
        z2 = [sb.tile([128, NI * A_], f32, tag=f"z2_{jh}", name=f"z2_{jh}")
              for jh in range(2)]
        z2_r = [z2[jh][:].rearrange("p (i a) -> p i a", i=NI)
                for jh in range(2)]
        colof = lambda k: (k % 2) * 512 + (k // 2) * 128

        def emit_scores2(il, bh, Jh, jh):
            # 8 heads x [J' = half Jh (128 keys), j = half jh (128 queries)]
            sps = pss.tile([128, 1024], f32, tag="s", name="sps2")
            for k in range(8):
                a = PERM[bh * 8 + k]
                c, s4 = a // 4, a % 4
                tp = (96, 0) if s4 == 3 else None
                nc.tensor.matmul(
                    sps[:, colof(k):colof(k) + 128],
                    q2_r[32 * s4:32 * s4 + 16, 4 + c, Jh, :, :, il],
                    q2_r[32 * s4:32 * s4 + 16, c, jh, :, :, il],
                    start=True, stop=True, tile_position=tp)
            return sps

        # per-il tail: normalize + transpose to feature-major + FC + store
        def il_tail(il):
            pa_f = [sb.tile([128, NP], f32r, tag=f"paf{ah}", name=f"paf{ah}",
                            bufs=2) for ah in range(2)]
            for jh in range(2):
                zv = z2_r[jh][:, il, :]
                nc.vector.reciprocal(zv, zv)
                # DVE, not Pool: the Q7 launch overhead (~1us) would sit on
                # every il's FC chain and on the program tail
                nc.vector.tensor_tensor(
                    pa_r[jh][:, il, :, :], pa_r[jh][:, il, :, :],
                    zv.rearrange("p a -> p a ()").broadcast_to(
                        [128, A_, B_]), op=MUL)
            for ah in range(2):
                ps = psv.tile([128, 512], f32, tag="w", bufs=2, name="pf")
                for jh in range(2):
                    nc.tensor.transpose(
                        ps[:, jh * 128:jh * 128 + 128],
                        pa_tok[jh][:, il * F + ah * 128:
                                   il * F + (ah + 1) * 128],
                        ident[:])
                nc.vector.tensor_copy(pa_f[ah][:, 0:256], ps[:, 0:256])
            for jh in range(2):
                ps = psv.tile([128, 512], f32, tag="w", bufs=2, name="fc")
                for kt in range(2):
                    nc.tensor.matmul(
                        ps[:, 0:F],
                        pa_f[kt][:, jh * 128:(jh + 1) * 128],
                        wc_sb[:, kt * F:(kt + 1) * F],
                        start=(kt == 0), stop=(kt == 1))
                ot = outp.tile([128, F], f32, tag="ot")
                nc.vector.tensor_tensor(ot[:], ps[:, 0:F], bias_sb[:], op=ADD)
                nc.sync.dma_start(out_d[il, jh * 128:(jh + 1) * 128, :], ot[:])

        # task kinds: (Jh keys, jh queries); every kind is a full start/stop
        # AV group; partial sums combine in SBUF (copy then add)
        KINDS = {"q0": (0, 0),   # -> copy to pa0
                 "q1": (1, 0),   # -> add to pa0
                 "c0": (0, 1),   # -> copy to pa1
                 "c1": (1, 1)}   # -> add to pa1

        tps_map = {}

        def process_b(il, bh, kind, sps):
            Jh, jh = KINDS[kind]
            if bh == 0:
                tps_map[(il, kind)] = psv.tile([128, 512], f32, tag="v",
                                               bufs=2, name="tps2")
            tps = tps_map[(il, kind)]
            aex = expp.tile([128, 1024], bf16, tag="aex2", name="aex2")
            nc.scalar.activation(aex[:], sps[:], Exp)
            for k in range(8):
                a = PERM[bh * 8 + k]
                nc.tensor.matmul(
                    tps[:, bh * 136 + k * 17: bh * 136 + k * 17 + 17],
                    aex[:, colof(k):colof(k) + 128],
                    v2t_r[Jh][:, il, a, :], start=True, stop=True)
            if bh == 1:
                jh_dst = jh
                tr = tps_map.pop((il, kind))[:, 0:272] \
                    .rearrange("p (s c) -> p s c", s=16)
                if kind in ("q0", "c0"):
                    nc.vector.tensor_copy(pa_r[jh_dst][:, il, :, :],
                                          tr[:, :, 0:16])
                    nc.vector.tensor_copy(z2_r[jh_dst][:, il, :], tr[:, :, 16])
                else:
                    dst = pa_r[jh_dst][:, il, :, :]
                    nc.vector.tensor_tensor(dst, dst, tr[:, :, 0:16], op=ADD)
                    zdst = z2_r[jh_dst][:, il, :]
                    nc.vector.tensor_tensor(zdst, zdst, tr[:, :, 16], op=ADD)
                    if kind == "c1":
                        il_tail(il)

        # ---- half 0 mixes, then the [J0, j0] quadrant while A2A#2 flies.
        # Half 1's t2 loads issue early (SP blocks harmlessly); its mixes
        # must NOT interleave into the quadrant: their matmuls wait on
        # A2A#2 and would stall the in-order PE queue. ----
        load_and_mix(0)
        _t2load(1)
        prev = None
        for il in range(NI):
            for bh in range(2):
                sps = emit_scores2(il, bh, 0, 0)
                if prev is not None:
                    process_b(*prev)
                prev = (il, bh, "q0", sps)
        process_b(*prev)
        prev = None
        for th in mix_thunks(1)[1:]:
            th()

        # ---- remaining three quadrants ----
        for il in range(NI):
            for kind in ("q1", "c0", "c1"):
                for bh in range(2):
                    sps = emit_scores2(il, bh, *KINDS[kind])
                    if prev is not None:
                        process_b(*prev)
                    prev = (il, bh, kind, sps)
        process_b(*prev)


# ---------------------------------------------------------------------------
# host side
# ---------------------------------------------------------------------------

def _pad_heads(w, n_in):
    """(n_in, F) with cols f=(a,b) -> (n_in, 4*128): chunk c holds a in
    [4c,4c+4) at col 32*(a%4)+b, cols +16..31 zero."""
    out = np.zeros((n_in, 4 * 128), dtype=np.float32)
    w = w.reshape(n_in, A_, B_)
    for a in range(A_):
        c, s4 = a // 4, a % 4
        out[:, c * 128 + 32 * s4: c * 128 + 32 * s4 + B_] = w[:, a, :]
    return out


def prep_inputs(x, W1, W2, fc1_w, fc1_b, fc2_w, fc2_b):
    """Host-side weight prep + per-core input maps."""
    x = np.asarray(x, dtype=np.float32)
    W1 = np.asarray(W1, dtype=np.float32)
    W2 = np.asarray(W2, dtype=np.float32)
    fc1_w = np.asarray(fc1_w, dtype=np.float32)
    fc1_b = np.asarray(fc1_b, dtype=np.float32)
    fc2_w = np.asarray(fc2_w, dtype=np.float32)
    fc2_b = np.asarray(fc2_b, dtype=np.float32)

    w1q = _pad_heads(W1[0].reshape(D, F), D)
    w1k = _pad_heads(W1[1].reshape(D, F), D)
    w1qk = np.concatenate([w1q, w1k], axis=1).astype(np.float16)
    w1v = W1[2].reshape(D, F).astype(np.float16)

    row_perm = np.array([PERM[pos] * B_ + b for pos in range(A_)
                         for b in range(B_)])
    w2q = _pad_heads(W2[0].reshape(F, F)[row_perm], F)
    w2k = _pad_heads(W2[1].reshape(F, F)[row_perm], F)
    w2qk = np.concatenate([w2q, w2k], axis=1).astype(np.float16)
    w2v = np.ascontiguousarray(W2[2].reshape(F, F)[row_perm]).astype(np.float16)

    wc = np.ascontiguousarray((fc1_w @ fc2_w)[row_perm]).astype(np.float32)
    bc = (fc1_b @ fc2_w + fc2_b).astype(np.float32)
    bc_rep = np.ascontiguousarray(np.broadcast_to(bc, (128, F)))

    in_maps = []
    for s in range(NCORE):
        in_maps.append({
            "x": np.ascontiguousarray(
                x[:, s * PC:(s + 1) * PC, :]).astype(np.float16),
            "w1qk": w1qk, "w1v": w1v,
            "w2qk": w2qk, "w2v": w2v,
            "wc": wc, "bc": bc_rep,
        })
    return in_maps


_CACHE = {}


def kernel(**inputs):
    from concourse.bass_utils import run_bass_kernel_spmd

    in_maps = prep_inputs(**inputs)
    if "nc" not in _CACHE:
        _CACHE["nc"] = build_program("AB", NCORE)
    nc = _CACHE["nc"]
    res = run_bass_kernel_spmd(nc, in_maps, list(range(NCORE)))
    out = np.empty((NF, NP, F), dtype=np.float32)
    for s in range(NCORE):
        out[s * NI:(s + 1) * NI, JPERM, :] = res.results[s]["out"]
    return out
